# revision 3
# baseline (speedup 1.0000x reference)
"""Trainium2 Bass kernel for DecoupledRadialAngularLoss, v2.

Vocab-parallel over 8 cores (V=50257 -> 8 x 6400 padded). All O(B*L*V)
math on device; host does O(input)-sized layout/normalize/cast prep only.

Per core:
  PE:  G[t,v] = (8 u_s).(8 u_w) = 64 cos   (fp8 DoubleRow, f-contraction)
       M[t,f] = sum_v p8[t,v] u_w8[v,f]    (fp8 DoubleRow, v-contraction)
  ACT: exp(G/64 - 1) + row accum -> Z partials (softmax denominator;
       cos<=1 so the fixed -1 shift replaces the max pass)
  DVE: affine_mul_reduce: sum_v p8*(alpha*bits(p8)+beta) -> Sp*ln p
       (fast-log on the fp8 bit pattern; alpha/beta host-calibrated by
       p-weighted least squares so quantizer bias cancels)
       tensor_tensor_reduce: B[t] = sum_f u_s16[t,f]*M[t,f] -> Sp*cos

Host combine: KL_row = A - B + 1 + ln(Z_row); radial terms O(B*L) on host.
"""

import math

import ml_dtypes
import numpy as np

import concourse.bass as bass
import concourse.mybir as mybir
import concourse.tile as tile
from concourse import bacc
from concourse import bass_utils

# ---- problem constants ----
B, L, N_FEAT = 2, 1024, 768
TOK = B * L
V = 50257
R_MAX = 3.0
LAMBDA_RADIAL = 0.1
T_TEMP = 1.0
LOG_V = math.log(V)

N_CORES = 8
VP = 6400                    # per-core padded vocab shard
V_PAD_TOTAL = N_CORES * VP   # 51200
N_PAD_LAST = V_PAD_TOTAL - V  # 943 zero columns on core 7

NT = TOK // 128              # 16 token tiles
NF2 = N_FEAT // 256          # 3 DoubleRow feature k-tile pairs
NVT = VP // 256              # 25 DoubleRow vocab k-tile pairs
SCS = [(o, min(1024, VP - o)) for o in range(0, VP, 1024)]  # 6x1024 + 256
NSC = len(SCS)               # 7

P_SCALE = float(2 ** 16)     # p -> fp8 prescale
US_SCALE = 8.0               # u_s -> fp8 prescale
UW_SCALE = 8.0               # u_w -> fp8 prescale
G_SCALE = US_SCALE * UW_SCALE          # G = 64 * cos
B_SCALE = P_SCALE * UW_SCALE           # M = P_SCALE*8 * (p @ u_w)

BF16 = mybir.dt.bfloat16
FP8 = mybir.dt.float8e4
U8 = mybir.dt.uint8
F32 = mybir.dt.float32
AF = mybir.ActivationFunctionType
ALU = mybir.AluOpType

_CACHE = {}


def _build_program():
    nc = bacc.Bacc("TRN2", target_bir_lowering=False, debug=False)

    h8_d = nc.dram_tensor("h8", (128, NF2, 2, TOK), FP8, kind="ExternalInput").ap()
    w8_d = nc.dram_tensor("w8", (128, NF2, 2, VP), FP8, kind="ExternalInput").ap()
    w8v_d = nc.dram_tensor("w8v", (128, NVT, 2, N_FEAT), FP8, kind="ExternalInput").ap()
    p8_d = nc.dram_tensor("p8", (NT, 128, VP), FP8, kind="ExternalInput").ap()
    p8t_d = nc.dram_tensor("p8t", (NT, 128, NVT, 2, 128), FP8, kind="ExternalInput").ap()
    u16_d = nc.dram_tensor("u16", (128, NT, N_FEAT), BF16, kind="ExternalInput").ap()
    ab_d = nc.dram_tensor("ab", (128, 2), F32, kind="ExternalInput").ap()

    z_d = nc.dram_tensor("Z", (128, NT * NSC), F32, kind="ExternalOutput").ap()
    a_d = nc.dram_tensor("A", (128, NT), F32, kind="ExternalOutput").ap()
    b_d = nc.dram_tensor("Bt", (128, NT), F32, kind="ExternalOutput").ap()

    with tile.TileContext(nc) as tc:
        with tc.tile_pool(name="persist", bufs=1) as persist:
            h8_sb = persist.tile([128, NF2, 2, TOK], FP8)
            w8_sb = persist.tile([128, NF2, 2, VP], FP8)
            w8v_sb = persist.tile([128, NVT, 2, N_FEAT], FP8)
            u16_sb = persist.tile([128, NT, N_FEAT], BF16)
            ab_sb = persist.tile([128, 2], F32)
            neg1 = persist.tile([128, 1], F32)
            zparts = persist.tile([128, NT * NSC], F32)
            aparts = persist.tile([128, NT], F32)
            bparts = persist.tile([128, NT], F32)

            nc.vector.memset(neg1, -1.0)
            nc.sync.dma_start(out=h8_sb, in_=h8_d)
            nc.sync.dma_start(out=w8_sb, in_=w8_d)
            nc.sync.dma_start(out=w8v_sb, in_=w8v_d)
            nc.sync.dma_start(out=u16_sb, in_=u16_d)
            nc.sync.dma_start(out=ab_sb, in_=ab_d)

            with (
                tc.tile_pool(name="stream", bufs=3) as stream,
                tc.tile_pool(name="scratch", bufs=2) as scratch,
                tc.tile_pool(name="pg", bufs=3, space="PSUM") as pg,
                tc.tile_pool(name="pm", bufs=1, space="PSUM") as pm,
            ):
                for j in range(NT):
                    p8_sb = stream.tile([128, VP], FP8, tag="p8")
                    nc.sync.dma_start(out=p8_sb, in_=p8_d[j])
                    p8t_sb = stream.tile([128, NVT, 2, 128], FP8, tag="p8t")
                    nc.sync.dma_start(out=p8t_sb, in_=p8t_d[j])

                    # ---- B-GEMM: M[t,f] = sum_v p8_T[v,t] * w8v[v,f] ----
                    M = pm.tile([128, N_FEAT], F32, tag="M")
                    for vt in range(NVT):
                        st = p8t_sb[:, vt, :, :]
                        for c in (0, 512):
                            cw = min(512, N_FEAT - c)
                            nc.tensor.matmul(
                                M[:, c:c + cw], st, w8v_sb[:, vt, :, c:c + cw],
                                start=(vt == 0), stop=(vt == NVT - 1),
                                perf_mode=mybir.MatmulPerfMode.DoubleRow)
                    ttr_out = scratch.tile([128, N_FEAT], BF16, tag="ttro")
                    nc.vector.tensor_tensor_reduce(
                        out=ttr_out, in0=M, in1=u16_sb[:, j],
                        scale=1.0 / B_SCALE, scalar=0.0,
                        op0=ALU.mult, op1=ALU.add,
                        accum_out=bparts[:, j:j + 1])

                    # ---- A: sum_v p8 * (alpha*bits(p8) + beta) ----
                    amr_out = scratch.tile([128, VP], BF16, tag="amro")
                    nc.vector.affine_mul_reduce(
                        out=amr_out, accum_out=aparts[:, j:j + 1],
                        in0=p8_sb.bitcast(U8), in1=p8_sb,
                        scale=ab_sb[:, 0:1], bias=ab_sb[:, 1:2])

                    # ---- main GEMM + exp ----
                    if stage == "dmaonly":
                        if j == 0:
                            nc.vector.memset(zparts, 1.0)
                        continue
                    for s, (off, scw) in enumerate(SCS):
                        G = pg.tile([128, 1024], F32, tag="G")
                        for t in range(NF2):
                            st = h8_sb[:, t, :, j * 128:(j + 1) * 128]
                            for c in range(0, scw, 512):
                                cw = min(512, scw - c)
                                nc.tensor.matmul(
                                    G[:, c:c + cw], st,
                                    w8_sb[:, t, :, off + c:off + c + cw],
                                    start=(t == 0), stop=(t == NF2 - 1),
                                    perf_mode=mybir.MatmulPerfMode.DoubleRow)
                        exp_out = scratch.tile([128, 1024], BF16, tag="expo", bufs=3)
                        nc.scalar.activation(
                            out=exp_out[:, :scw], in_=G[:, :scw], func=AF.Exp,
                            scale=1.0 / G_SCALE, bias=neg1,
                            accum_out=zparts[:, j * NSC + s:j * NSC + s + 1])

                nc.sync.dma_start(out=z_d, in_=zparts)
                nc.sync.dma_start(out=a_d, in_=aparts)
                nc.sync.dma_start(out=b_d, in_=bparts)

    nc.compile()
    return nc


def _calibrate_fastlog(p_flat: np.ndarray):
    """Fit alpha, beta so that sum p8*(alpha*bits(p8)+beta) ~= sum p*ln(p).

    WLS with weights w = p8 (the device-side multiplicand) against target
    y = p*ln(p)/p8 zeroes the weighted residual mean, so the fp8
    quantization bias of p itself is absorbed into (alpha, beta)."""
    s = p_flat[:: max(1, p_flat.size // 2_000_000)].astype(np.float64)
    q = (s * P_SCALE).astype(ml_dtypes.float8_e4m3)
    bits = q.view(np.uint8).astype(np.float64)
    s8 = q.astype(np.float64) / P_SCALE
    mask = s8 > 0
    bits, s, s8 = bits[mask], s[mask], s8[mask]
    w = s8
    y = s * np.log(s) / s8
    sw = w.sum()
    mx = (w * bits).sum() / sw
    my = (w * y).sum() / sw
    cov = (w * (bits - mx) * (y - my)).sum()
    var = (w * (bits - mx) ** 2).sum()
    alpha = cov / var
    beta = my - alpha * mx
    return float(alpha), float(beta)


def _get_program():
    if "nc" not in _CACHE:
        _CACHE["nc"] = _build_program()
    return _CACHE["nc"]


def _prep_inputs(h_student, W_vocab, p_teacher):
    """Host-side shard/layout prep (numpy, O(input size))."""
    FP8NP = ml_dtypes.float8_e4m3

    sp_s = h_student.reshape(TOK, N_FEAT + 1)[:, 1:].astype(np.float32)
    u_s = sp_s / np.linalg.norm(sp_s, axis=1, keepdims=True)
    u_s8 = (US_SCALE * u_s).astype(FP8NP)
    # h8[p, t, q, tok] = u_s8[tok, t*256 + q*128 + p]
    h8 = np.ascontiguousarray(
        u_s8.T.reshape(NF2, 2, 128, TOK).transpose(2, 0, 1, 3))

    sp_w = W_vocab[:, 1:].astype(np.float32)
    u_w = sp_w / np.linalg.norm(sp_w, axis=1, keepdims=True)
    u_w8_full = np.zeros((V_PAD_TOTAL, N_FEAT), dtype=FP8NP)
    u_w8_full[:V] = (UW_SCALE * u_w).astype(FP8NP)

    p32 = p_teacher.reshape(TOK, V).astype(np.float32)
    p8_full = np.zeros((TOK, V_PAD_TOTAL), dtype=FP8NP)
    p8_full[:, :V] = (P_SCALE * p32).astype(FP8NP)

    # u16[p, j, f] = u_s[j*128+p, f]
    u16 = np.ascontiguousarray(
        u_s.astype(ml_dtypes.bfloat16).reshape(NT, 128, N_FEAT).transpose(1, 0, 2))

    alpha, beta = _calibrate_fastlog(p32.reshape(-1))
    ab = np.tile(np.array([[alpha, beta]], dtype=np.float32), (128, 1))

    in_maps = []
    for k in range(N_CORES):
        lo, hi = k * VP, (k + 1) * VP
        w8s = u_w8_full[lo:hi]
        # w8[p, t, q, v] = w8s[v, t*256+q*128+p]
        w8 = np.ascontiguousarray(
            w8s.T.reshape(NF2, 2, 128, VP).transpose(2, 0, 1, 3))
        # w8v[p, vt, q, f] = w8s[vt*256+q*128+p, f]
        w8v = np.ascontiguousarray(
            w8s.reshape(NVT, 2, 128, N_FEAT).transpose(2, 0, 1, 3))
        p8s = p8_full[:, lo:hi]
        p8 = np.ascontiguousarray(p8s.reshape(NT, 128, VP))
        # p8t[j, p, vt, q, m] = p8s[j*128+m, vt*256+q*128+p]
        p8t = np.ascontiguousarray(
            p8s.reshape(NT, 128, NVT, 2, 128).transpose(0, 4, 2, 3, 1))
        in_maps.append({"h8": h8, "w8": w8, "w8v": w8v,
                        "p8": p8, "p8t": p8t, "u16": u16, "ab": ab})
    return in_maps


def _combine(results, h_student, teacher_entropy):
    """Host-side gather of per-core row partials + tiny radial part."""
    def pm_to_tok(arr, ncol):  # [128, NT*ncol] -> [TOK(, ncol)]
        a = arr.reshape(128, NT, ncol).transpose(1, 0, 2)  # [j, p, ncol]
        return np.ascontiguousarray(a).reshape(TOK, ncol)

    Z = np.zeros(TOK, np.float64)
    A = np.zeros(TOK, np.float64)
    Bp = np.zeros(TOK, np.float64)
    for k in range(N_CORES):
        Z += pm_to_tok(results[k]["Z"].astype(np.float64), NSC).sum(axis=1)
        A += pm_to_tok(results[k]["A"].astype(np.float64), 1)[:, 0]
        Bp += pm_to_tok(results[k]["Bt"].astype(np.float64), 1)[:, 0]
    A /= P_SCALE

    # padded vocab columns on core 7 contribute exp(0 - 1) each to Z
    Z -= N_PAD_LAST * math.exp(-1.0)

    logZ = 1.0 + np.log(Z)
    kl_rows = A - Bp + logZ
    kl = kl_rows.sum() / TOK
    l_angular = kl * (T_TEMP ** 2)

    x0 = np.clip(h_student.reshape(TOK, N_FEAT + 1)[:, 0].astype(np.float64),
                 1.0 + 1e-7, None)
    r_s = np.arccosh(x0)
    H_norm = np.clip(teacher_entropy.reshape(TOK).astype(np.float64) / LOG_V,
                     0.0, 1.0)
    r_target = (1.0 / (1.0 + np.exp(H_norm))) * R_MAX
    l_radial = np.mean((r_s - r_target) ** 2)
    l_total = l_angular + LAMBDA_RADIAL * l_radial

    return np.array([l_total, l_angular, l_radial,
                     r_s.mean(), r_target.mean(), H_norm.mean()],
                    dtype=np.float32)


def kernel(h_student, W_vocab, p_teacher, teacher_entropy):
    in_maps = _prep_inputs(h_student, W_vocab, p_teacher)
    nc = _get_program()
    res = bass_utils.run_bass_kernel_spmd(nc, in_maps,
                                          core_ids=list(range(N_CORES)))
    return _combine(res.results, h_student, teacher_entropy)


# revision 4
# speedup vs baseline: 2.2500x; 2.2500x over previous
"""Trainium2 Bass kernel for DecoupledRadialAngularLoss, v3.

Vocab-parallel over 8 cores (V=50257 -> 8 x 6400 padded). All O(B*L*V)
math on device; host does O(input)-sized layout/normalize/cast prep only.

Per core, per 128-token tile j and 1536-wide vocab chunk:
  PE:  G = (8 u_s).(8 u_w) = 64*cos        (fp8 DoubleRow GEMM)
  ACT: exp(G/64 - 1) + row accum -> Z partials (softmax denominator;
       cos<=1 so the fixed -1 shift replaces the max pass)
  DVE: affine_mul_reduce #1 (per tile): sum_v p8*(alpha*bits(p8)+beta)
       -> sum p*ln p  (fast-log on the fp8 bit pattern; alpha/beta are
       host-calibrated by p8-weighted least squares, which also absorbs
       the fp8 quantization bias of p)
       affine_mul_reduce #2 (per chunk): sum_v (G/64)*p8 -> sum p*cos

Host combine: KL_row = A - B + 1 + ln(Z_row); radial terms O(B*L) on host.
"""

import math

import ml_dtypes
import numpy as np

import concourse.bass as bass
import concourse.mybir as mybir
import concourse.tile as tile
from concourse import bacc
from concourse import bass_utils

# ---- problem constants ----
B, L, N_FEAT = 2, 1024, 768
TOK = B * L
V = 50257
R_MAX = 3.0
LAMBDA_RADIAL = 0.1
T_TEMP = 1.0
LOG_V = math.log(V)

N_CORES = 8
VP = 6400                    # per-core padded vocab shard
V_PAD_TOTAL = N_CORES * VP   # 51200
N_PAD_LAST = V_PAD_TOTAL - V  # 943 zero columns on core 7

NT = TOK // 128              # 16 token tiles
NF2 = N_FEAT // 256          # 3 DoubleRow feature k-tile pairs
SCS = [(o, min(1536, VP - o)) for o in range(0, VP, 1536)]  # 4x1536 + 256
NSC = len(SCS)               # 5

P_SCALE = float(2 ** 16)     # p -> fp8 prescale
US_SCALE = 8.0               # u_s -> fp8 prescale
UW_SCALE = 8.0               # u_w -> fp8 prescale
G_SCALE = US_SCALE * UW_SCALE          # G = 64 * cos

BF16 = mybir.dt.bfloat16
FP8 = mybir.dt.float8e4
U8 = mybir.dt.uint8
F32 = mybir.dt.float32
AF = mybir.ActivationFunctionType
ALU = mybir.AluOpType

_CACHE = {}


def _build_program(stage="full"):
    nc = bacc.Bacc("TRN2", target_bir_lowering=False, debug=False)

    h8_d = nc.dram_tensor("h8", (128, NF2, 2, TOK), FP8, kind="ExternalInput").ap()
    w8_d = nc.dram_tensor("w8", (128, NF2, 2, VP), FP8, kind="ExternalInput").ap()
    p8_d = nc.dram_tensor("p8", (NT, 128, VP), FP8, kind="ExternalInput").ap()
    ab_d = nc.dram_tensor("ab", (128, 2), F32, kind="ExternalInput").ap()

    z_d = nc.dram_tensor("Z", (128, NT * NSC), F32, kind="ExternalOutput").ap()
    a_d = nc.dram_tensor("A", (128, NT), F32, kind="ExternalOutput").ap()
    b_d = nc.dram_tensor("Bt", (128, NT * NSC), F32, kind="ExternalOutput").ap()

    reps = 9 if stage == "x9" else 1

    with tile.TileContext(nc) as tc:
        with tc.tile_pool(name="persist", bufs=1) as persist:
            h8_sb = persist.tile([128, NF2, 2, TOK], FP8)
            w8_sb = persist.tile([128, NF2, 2, VP], FP8)
            ab_sb = persist.tile([128, 2], F32)
            neg1 = persist.tile([128, 1], F32)
            zparts = persist.tile([128, NT * NSC], F32)
            aparts = persist.tile([128, NT], F32)
            bparts = persist.tile([128, NT * NSC], F32)

            nc.vector.memset(neg1, -1.0)
            if stage != "empty":
                nc.sync.dma_start(out=h8_sb, in_=h8_d)
                nc.sync.dma_start(out=w8_sb, in_=w8_d)
            nc.sync.dma_start(out=ab_sb, in_=ab_d)

            with (
                tc.tile_pool(name="stream", bufs=3) as stream,
                tc.tile_pool(name="scratch", bufs=2) as scratch,
                tc.tile_pool(name="pg", bufs=2, space="PSUM") as pg,
            ):
                if stage == "empty":
                    nc.vector.memset(zparts, 1.0)
                    nc.vector.memset(aparts, 0.0)
                    nc.vector.memset(bparts, 0.0)
                for rep in range(0 if stage == "empty" else reps):
                    for j in range(NT):
                        p8_sb = stream.tile([128, VP], FP8, tag="p8")
                        nc.sync.dma_start(out=p8_sb, in_=p8_d[j])

                        # ---- A: sum_v p8 * (alpha*bits(p8) + beta) ----
                        amr_out = scratch.tile([128, VP], BF16, tag="amro")
                        nc.vector.affine_mul_reduce(
                            out=amr_out, accum_out=aparts[:, j:j + 1],
                            in0=p8_sb.bitcast(U8), in1=p8_sb,
                            scale=ab_sb[:, 0:1], bias=ab_sb[:, 1:2])

                        # ---- main GEMM + exp + p*cos ----
                        for s, (off, scw) in enumerate(SCS):
                            G = pg.tile([128, 1536], F32, tag="G")
                            for t in range(NF2):
                                st = h8_sb[:, t, :, j * 128:(j + 1) * 128]
                                for c in range(0, scw, 512):
                                    cw = min(512, scw - c)
                                    nc.tensor.matmul(
                                        G[:, c:c + cw], st,
                                        w8_sb[:, t, :, off + c:off + c + cw],
                                        start=(t == 0), stop=(t == NF2 - 1),
                                        perf_mode=mybir.MatmulPerfMode.DoubleRow)
                            k = j * NSC + s
                            exp_out = scratch.tile([128, 1536], BF16,
                                                   tag="expo", bufs=3)
                            nc.scalar.activation(
                                out=exp_out[:, :scw], in_=G[:, :scw], func=AF.Exp,
                                scale=1.0 / G_SCALE, bias=neg1,
                                accum_out=zparts[:, k:k + 1])
                            pg_out = scratch.tile([128, 1536], BF16,
                                                  tag="pgo", bufs=3)
                            nc.vector.affine_mul_reduce(
                                out=pg_out[:, :scw], accum_out=bparts[:, k:k + 1],
                                in0=G[:, :scw], in1=p8_sb[:, off:off + scw],
                                scale=1.0 / G_SCALE, bias=0.0)

                nc.sync.dma_start(out=z_d, in_=zparts)
                nc.sync.dma_start(out=a_d, in_=aparts)
                nc.sync.dma_start(out=b_d, in_=bparts)

    nc.compile()
    return nc


def _calibrate_fastlog(p_flat: np.ndarray):
    """Fit alpha, beta so that sum p8*(alpha*bits(p8)+beta) ~= sum p*ln(p).

    WLS with weights w = p8 (the device-side multiplicand) against target
    y = p*ln(p)/p8 zeroes the weighted residual mean, so the fp8
    quantization bias of p itself is absorbed into (alpha, beta)."""
    s = p_flat[:: max(1, p_flat.size // 2_000_000)].astype(np.float64)
    q = (s * P_SCALE).astype(ml_dtypes.float8_e4m3)
    bits = q.view(np.uint8).astype(np.float64)
    s8 = q.astype(np.float64) / P_SCALE
    mask = s8 > 0
    bits, s, s8 = bits[mask], s[mask], s8[mask]
    w = s8
    y = s * np.log(s) / s8
    sw = w.sum()
    mx = (w * bits).sum() / sw
    my = (w * y).sum() / sw
    alpha = (w * (bits - mx) * (y - my)).sum() / (w * (bits - mx) ** 2).sum()
    beta = my - alpha * mx
    return float(alpha), float(beta)


def _get_program(stage="full"):
    key = "nc_" + stage
    if key not in _CACHE:
        _CACHE[key] = _build_program(stage)
    return _CACHE[key]


def _prep_inputs(h_student, W_vocab, p_teacher):
    """Host-side shard/layout prep (numpy, O(input size))."""
    FP8NP = ml_dtypes.float8_e4m3

    sp_s = h_student.reshape(TOK, N_FEAT + 1)[:, 1:].astype(np.float32)
    u_s = sp_s / np.linalg.norm(sp_s, axis=1, keepdims=True)
    u_s8 = (US_SCALE * u_s).astype(FP8NP)
    # h8[p, t, q, tok] = u_s8[tok, t*256 + q*128 + p]
    h8 = np.ascontiguousarray(
        u_s8.T.reshape(NF2, 2, 128, TOK).transpose(2, 0, 1, 3))

    sp_w = W_vocab[:, 1:].astype(np.float32)
    u_w = sp_w / np.linalg.norm(sp_w, axis=1, keepdims=True)
    u_w8_full = np.zeros((V_PAD_TOTAL, N_FEAT), dtype=FP8NP)
    u_w8_full[:V] = (UW_SCALE * u_w).astype(FP8NP)

    p32 = p_teacher.reshape(TOK, V).astype(np.float32)
    p8_full = np.zeros((TOK, V_PAD_TOTAL), dtype=FP8NP)
    p8_full[:, :V] = (P_SCALE * p32).astype(FP8NP)

    alpha, beta = _calibrate_fastlog(p32.reshape(-1))
    ab = np.tile(np.array([[alpha, beta]], dtype=np.float32), (128, 1))

    in_maps = []
    for k in range(N_CORES):
        lo, hi = k * VP, (k + 1) * VP
        w8s = u_w8_full[lo:hi]
        # w8[p, t, q, v] = w8s[v, t*256+q*128+p]
        w8 = np.ascontiguousarray(
            w8s.T.reshape(NF2, 2, 128, VP).transpose(2, 0, 1, 3))
        p8 = np.ascontiguousarray(p8_full[:, lo:hi].reshape(NT, 128, VP))
        in_maps.append({"h8": h8, "w8": w8, "p8": p8, "ab": ab})
    return in_maps


def _combine(results, h_student, teacher_entropy):
    """Host-side gather of per-core row partials + tiny radial part."""
    def pm_to_tok(arr, ncol):  # [128, NT*ncol] -> [TOK, ncol]
        a = arr.reshape(128, NT, ncol).transpose(1, 0, 2)  # [j, p, ncol]
        return np.ascontiguousarray(a).reshape(TOK, ncol)

    Z = np.zeros(TOK, np.float64)
    A = np.zeros(TOK, np.float64)
    Bp = np.zeros(TOK, np.float64)
    for k in range(N_CORES):
        Z += pm_to_tok(results[k]["Z"].astype(np.float64), NSC).sum(axis=1)
        A += pm_to_tok(results[k]["A"].astype(np.float64), 1)[:, 0]
        Bp += pm_to_tok(results[k]["Bt"].astype(np.float64), NSC).sum(axis=1)
    A /= P_SCALE
    Bp /= P_SCALE

    # padded vocab columns on core 7 contribute exp(0 - 1) each to Z
    Z -= N_PAD_LAST * math.exp(-1.0)

    logZ = 1.0 + np.log(Z)
    kl_rows = A - Bp + logZ
    kl = kl_rows.sum() / TOK
    l_angular = kl * (T_TEMP ** 2)

    x0 = np.clip(h_student.reshape(TOK, N_FEAT + 1)[:, 0].astype(np.float64),
                 1.0 + 1e-7, None)
    r_s = np.arccosh(x0)
    H_norm = np.clip(teacher_entropy.reshape(TOK).astype(np.float64) / LOG_V,
                     0.0, 1.0)
    r_target = (1.0 / (1.0 + np.exp(H_norm))) * R_MAX
    l_radial = np.mean((r_s - r_target) ** 2)
    l_total = l_angular + LAMBDA_RADIAL * l_radial

    return np.array([l_total, l_angular, l_radial,
                     r_s.mean(), r_target.mean(), H_norm.mean()],
                    dtype=np.float32)


def kernel(h_student, W_vocab, p_teacher, teacher_entropy):
    in_maps = _prep_inputs(h_student, W_vocab, p_teacher)
    nc = _get_program()
    res = bass_utils.run_bass_kernel_spmd(nc, in_maps,
                                          core_ids=list(range(N_CORES)))
    return _combine(res.results, h_student, teacher_entropy)
